# revision 1
# baseline (speedup 1.0000x reference)
"""Trainium2 Bass kernel for nn_Decoder (4-layer transformer decoder).

Sharding: 8 cores = 4 batches x 2 token-halves. Core (b, r) owns token blocks
{r, r+2, r+4, r+6} (128 tokens each, interleaved for causal load balance).

Per-call input bytes are minimized: every weight is streamed as a distinct
1/8 slice (bf16) and reassembled on-device with 8-way AllGathers into
per-chunk DRAM buffers (chunked so layer-0 weights arrive first and later
layers gather behind compute). Encoder outputs arrive as per-pair halves and
are pair-gathered once. Per layer, the residual stream x is pair-exchanged
(1 small AllGather) and K/V are recomputed locally from the gathered x —
cheaper than exchanging K and V.

Weight AllGathers run on the Pool (gpsimd) queue; the small latency-critical
x/enc exchanges run on the SP (sync) queue so they never wait behind a big
weight gather.

Layout: activations transposed (xT: [DM on partitions, tokens free]).
All matmuls bf16 (f32 PSUM accumulate); residual stream f32r. Per-token
stats (layernorm, softmax denominator) via ones-matmuls on the PE.

Self-attention causal structure is made core-uniform by padding each key
block's query window to start at J0[kb]; the first 128-col slab of each
window gets a host-supplied 0/1 multiplicative mask (applied after exp).
"""

import math

import numpy as np
import ml_dtypes

# Problem constants (hardcoded; must match the harness problem).
L, DM, H, DK, DV, DFF = 4, 1024, 16, 64, 64, 4096
B, T = 4, 1024
EPS = 1e-5

P = 128
TOK = 512                      # tokens owned per core
ND = DM // P                   # 8 dm partition-tiles
NKB = T // P                   # 8 key blocks
NTB = TOK // P                 # 4 own token blocks
HP = H // 2                    # 8 head pairs
NF = DFF // P                  # 32 ffn row tiles
J0 = [max(0, math.ceil((kb - 1) / 2)) for kb in range(NKB)]
GPOS = [kb // 2 if kb % 2 == 0 else 4 + kb // 2 for kb in range(NKB)]

M1 = DM * DM
PIECE_ELEMS = {"self_Wq": M1, "self_Wk": M1, "self_Wv": M1, "self_Wo": M1,
               "cross_Wq": M1, "cross_Wk": M1, "cross_Wv": M1,
               "cross_Wo": M1, "ffn_W1": 4 * M1, "ffn_W2": 4 * M1}
_O10 = ["self_Wq", "self_Wk", "self_Wv", "cross_Wk", "cross_Wv", "self_Wo",
        "cross_Wq", "cross_Wo", "ffn_W1", "ffn_W2"]
# One AllGather per chunk, all on the gpsimd queue (NRT requires a single
# deterministic collective order). Emission order = need order; AG_SCHED maps
# layer -> chunks emitted right after that layer's x pair-exchange, so the
# small latency-critical exchange is never stuck behind a bulk gather.
CHUNKS = [
    [(w, 0) for w in _O10],
    [(w, 1) for w in _O10],
    [(w, 2) for w in _O10],
    [(w, 3) for w in _O10],
]
AG_SCHED = {0: [0, 1], 1: [2], 2: [3], 3: []}
CHUNK_ELEMS = [sum(PIECE_ELEMS[n] for n, _ in ch) for ch in CHUNKS]
WSLICE = sum(CHUNK_ELEMS) // 8
# (chunk index, offset within chunk) for each (name, layer) piece
PIECE_AT = {}
for _c, _ch in enumerate(CHUNKS):
    _off = 0
    for _n, _l in _ch:
        PIECE_AT[(_n, _l)] = (_c, _off)
        _off += PIECE_ELEMS[_n]

_BUILT = {}


def _build(num_devices=8, self_causal=True):
    import os
    dbg_no_pair = bool(os.environ.get("DBG_NO_PAIR"))
    dbg_no_w = bool(os.environ.get("DBG_NO_W"))
    dbg_no_chain = bool(os.environ.get("DBG_NO_CHAIN"))
    import concourse.bass as bass
    import concourse.tile as tile
    from concourse import bacc, mybir
    from contextlib import ExitStack

    dt = mybir.dt
    f32, f32r, bf16 = dt.float32, dt.float32r, dt.bfloat16
    AF = mybir.ActivationFunctionType
    OP = mybir.AluOpType
    RGP = [[0, 1], [2, 3], [4, 5], [6, 7]]
    RG8 = [list(range(8))]

    from concourse.bass import _add_dep_helper

    nc = bacc.Bacc("TRN2", target_bir_lowering=False, debug=False,
                   num_devices=num_devices)
    prev_cc = [None]

    def chain(cc):
        # NOTE: explicit cc->cc sync edges crash NRT (it owns collective
        # serialization); ordering is enforced via touch() data deps instead.
        return cc

    # ---- I/O ----
    xT_ext = nc.dram_tensor("xT", [DM, TOK], bf16, kind="ExternalInput").ap()
    encT_ext = nc.dram_tensor("encT", [DM, TOK], bf16,
                              kind="ExternalInput").ap()
    smask_ext = nc.dram_tensor("smask", [NKB, P, P], bf16,
                               kind="ExternalInput").ap()
    wsl_ext = nc.dram_tensor("wslice", [WSLICE], bf16,
                             kind="ExternalInput").ap()
    yT_ext = nc.dram_tensor("yT", [DM, TOK], bf16,
                            kind="ExternalOutput").ap()

    with tile.TileContext(nc) as tc, ExitStack() as stack:
        pers = stack.enter_context(tc.tile_pool(name="pers", bufs=1))
        dram = stack.enter_context(tc.tile_pool(name="dram", bufs=1,
                                                space="DRAM"))

        # ---- weight redistribution: 8-way chunk AllGathers (gpsimd queue) ----
        walls = [dram.tile([n], bf16, tag=f"wall{c}", name=f"wall{c}")
                 for c, n in enumerate(CHUNK_ELEMS)]
        _off8 = [0]
        for n in CHUNK_ELEMS[:-1]:
            _off8.append(_off8[-1] + n // 8)

        last_out = [None]

        def touch(bnc_slice, fix_src):
            """Order collectives without cc->cc sync edges: write 1 elem of
            the next collective's input from the previous one's output (data
            dep), then restore the real value."""
            if last_out[0] is not None:
                nc.sync.dma_start(bnc_slice, last_out[0])
                nc.sync.dma_start(bnc_slice, fix_src)

        def emit_wchunk(c):
            # collectives cannot read IO tensors: bounce the slice to
            # internal DRAM first
            n = CHUNK_ELEMS[c]
            off8 = _off8[c]
            bnc = dram.tile([n // 8], bf16, tag=f"wbnc{c}", name=f"wbnc{c}")
            nc.sync.dma_start(bnc[:], wsl_ext[off8:off8 + n // 8])
            touch(bnc[0:1], wsl_ext[off8:off8 + 1])
            if num_devices == 1 or dbg_no_w:
                for r in range(8):
                    nc.sync.dma_start(
                        walls[c][r * (n // 8):(r + 1) * (n // 8)], bnc[:])
            else:
                chain(nc.gpsimd.collective_compute(
                    "AllGather", mybir.AluOpType.bypass, replica_groups=RG8,
                    ins=[bnc[:].opt()],
                    outs=[walls[c][:].opt()]))
            last_out[0] = walls[c][0:1]

        def pair_ag(src_ap, dst, nm, touch_dst=None, fix_src=None):
            if touch_dst is not None:
                touch(touch_dst, fix_src)
            if num_devices == 1 or dbg_no_pair:
                for s in range(2):
                    nc.sync.dma_start(dst[s * DM:(s + 1) * DM], src_ap)
            else:
                chain(nc.gpsimd.collective_compute(
                    "AllGather", mybir.AluOpType.bypass, replica_groups=RGP,
                    ins=[src_ap.opt()], outs=[dst[:].opt()]))
            last_out[0] = dst[0:1, 0:1]

        def wview(name, l):
            c, off = PIECE_AT[(name, l)]
            n = PIECE_ELEMS[name]
            cols = DM if name != "ffn_W1" else DFF
            return walls[c][off:off + n].rearrange(
                "(o p m) -> p o m", p=P, m=cols)

        # preamble gathers: layer-0 QKV first, then x exchange, enc, rest
        enc_g = dram.tile([2 * DM, TOK], bf16, tag="encg", name="encg")
        xg_d0 = dram.tile([2 * DM, TOK], bf16, tag="xgd", bufs=2, name="xgd0")
        xT_b = dram.tile([DM, TOK], bf16, tag="xTb", name="xTb")
        nc.sync.dma_start(xT_b[:], xT_ext)
        enc_b = dram.tile([DM, TOK], bf16, tag="encb", name="encb")
        nc.sync.dma_start(enc_b[:], encT_ext)
        emit_wchunk(AG_SCHED[0][0])
        pair_ag(xT_b[:], xg_d0, "xg0",
                touch_dst=xT_b[0:1, 0:1], fix_src=xT_ext[0:1, 0:1])
        pair_ag(enc_b[:], enc_g, "enc",
                touch_dst=enc_b[0:1, 0:1], fix_src=encT_ext[0:1, 0:1])
        for c in AG_SCHED[0][1:]:
            emit_wchunk(c)

        # ---- constants ----
        ones_col_f = pers.tile([P, 1], f32, tag="ones_col_f")
        nc.vector.memset(ones_col_f[:], 1.0)
        ones_col = pers.tile([P, 1], f32r, tag="ones_col")
        nc.scalar.copy(ones_col[:], ones_col_f[:])
        ones_row_f = pers.tile([1, P], f32, tag="ones_row_f")
        nc.vector.memset(ones_row_f[:], 1.0)
        ones_row = pers.tile([1, P], f32r, tag="ones_row")
        nc.scalar.copy(ones_row[:], ones_row_f[:])
        eps_t = pers.tile([1, 1], f32, tag="eps_t")
        nc.vector.memset(eps_t[:], EPS)

        smask_sb = pers.tile([P, NKB, P], bf16, tag="smask")
        nc.sync.dma_start(smask_sb[:], smask_ext.rearrange("k p q -> p k q"))

        # enc resident in SBUF, global token order [P, ND, T]
        enc_sb = pers.tile([P, ND, T], bf16, tag="enc")
        for s in range(2):
            nc.sync.dma_start(
                enc_sb[:, :, s * TOK:(s + 1) * TOK],
                enc_g[s * DM:(s + 1) * DM].rearrange("(o p) t -> p o t", p=P))

        x_cur = None

        def load_whb(ph, name, l, half, nm):
            """[rows=1024, 1024] bf16 weight half -> [128, ND, 512] tile."""
            w = ph.tile([P, ND, TOK], bf16, tag="whb", bufs=2,
                        name=f"wb_{nm}")
            src = wview(name, l)
            for d in range(ND):
                nc.sync.dma_start(
                    w[:, d, :], src[:, d, half * TOK:(half + 1) * TOK])
            return w

        def cast_xb(ph, nm):
            """x_cur -> bf16 copy (DVE)."""
            xb = ph.tile([P, ND, TOK], bf16, tag="xb", bufs=1, name=f"xb_{nm}")
            for m in range(ND):
                nc.vector.tensor_copy(xb[:, m, :], x_cur[:, m, :])
            return xb

        def q_proj(ph, pools, xin, name, l, nm):
            qt = pers.tile([P, ND, TOK], bf16, tag="qt", name=f"qt_{nm}")
            for half in range(2):
                wq = load_whb(ph, name, l, half, f"q{nm}{half}")
                for m in range(4):
                    ps = pools.tile([P, TOK], f32, tag="proj", bufs=2,
                                    name=f"qps_{nm}{half}{m}")
                    for d in range(ND):
                        nc.tensor.matmul(
                            ps[:], wq[:, d, m * P:(m + 1) * P], xin[:, d, :],
                            start=(d == 0), stop=(d == ND - 1))
                    nc.vector.tensor_copy(qt[:, half * 4 + m, :], ps[:])
            return qt

        def kv_proj(ph, pools, kname, vname, l, get_k_in, get_v_in, kt, vg,
                    nm, after=None):
            """Full-token K^T [P,HP,2,TOK] and V [P,NKB,H,DV+1] from bf16
            input slices. get_k_in(d, s) -> [P, TOK]; get_v_in(d, c) ->
            [P, P] (block c in storage order)."""
            for half in range(2):
                wk = load_whb(ph, kname, l, half, f"k{nm}{half}")
                for s in range(2):
                    for m in range(4):
                        ps = pools.tile([P, TOK], f32, tag="proj", bufs=2,
                                        name=f"kps_{nm}{half}{s}{m}")
                        for d in range(ND):
                            mm = nc.tensor.matmul(
                                ps[:], wk[:, d, m * P:(m + 1) * P],
                                get_k_in(d, s),
                                start=(d == 0), stop=(d == ND - 1))
                            if after is not None:
                                _add_dep_helper(mm.ins, after[0].ins,
                                                sync=True,
                                                reason="crosskv after attn")
                                after = None
                        nc.vector.tensor_copy(kt[:, half * 4 + m, s, :],
                                              ps[:])
                wv = load_whb(ph, vname, l, half, f"v{nm}{half}")
                for c in range(NKB):
                    ps = pools.tile([P, TOK], f32, tag="proj", bufs=2,
                                    name=f"vps_{nm}{half}{c}")
                    for d in range(ND):
                        nc.tensor.matmul(
                            ps[:], get_v_in(d, c), wv[:, d, :],
                            start=(d == 0), stop=(d == ND - 1))
                    nc.vector.tensor_copy(
                        vg[:, c, half * 8:(half + 1) * 8, 0:DV],
                        ps.rearrange("p (h v) -> p h v", h=8))
            nc.vector.memset(vg[:, :, :, DV:DV + 1], 1.0)

        def attention(ph, aps, qt, ktg, vg, masked, nm):
            """K/V in SBUF -> normalized ctx_sb [P, ND, TOK] bf16."""
            ctx_sb = pers.tile([P, ND, TOK], bf16, tag="ctxs", name=f"ctx_{nm}")
            last_mm = [None]
            for p in range(HP):
                cps = [aps.tile([DV + 1, TOK], f32, tag="ctxps", bufs=2,
                                name=f"cps_{nm}{p}{h}") for h in range(2)]
                for kb in range(NKB):
                    qo = J0[kb] * P if masked else 0
                    c = GPOS[kb]
                    es = ph.tile([P, 2, TOK], bf16, tag="es", bufs=3,
                                 name=f"es_{nm}{p}{kb}")
                    for h in range(2):
                        sc = aps.tile([P, TOK], f32, tag="sc", bufs=3,
                                      name=f"sc_{nm}{p}{kb}{h}")
                        nc.tensor.matmul(
                            sc[:, qo:],
                            ktg[h * DV:(h + 1) * DV, p, c // 4,
                                (c % 4) * P:(c % 4 + 1) * P],
                            qt[h * DV:(h + 1) * DV, p, qo:],
                            start=True, stop=True)
                        nc.scalar.activation(
                            es[:, h, qo:], sc[:, qo:],
                            AF.Exp, scale=1.0 / math.sqrt(DK))
                    if masked:
                        nc.vector.tensor_tensor(
                            es[:, :, qo:qo + P], es[:, :, qo:qo + P],
                            smask_sb[:, kb, None, :].to_broadcast([P, 2, P]),
                            OP.mult)
                    for h in range(2):
                        last_mm[0] = nc.tensor.matmul(
                            cps[h][:, qo:], vg[:, c, 2 * p + h, :],
                            es[:, h, qo:], start=(kb == 0),
                            stop=(kb == NKB - 1))
                for h in range(2):
                    rec = pers.tile([1, TOK], f32r, tag="rec", bufs=2,
                                    name=f"rec_{nm}{p}{h}")
                    with nc.allow_low_precision(reason="f32r softmax denom"):
                        nc.vector.reciprocal(rec[:], cps[h][DV:DV + 1, :])
                    bc = aps.tile([P, TOK], f32, tag="bcps", bufs=1,
                                  name=f"bc_{nm}{p}{h}")
                    nc.tensor.matmul(bc[:], ones_row[:], rec[:],
                                     start=True, stop=True)
                    nc.vector.tensor_copy(ctx_sb[h * DV:(h + 1) * DV, p, :],
                                          cps[h][0:DV, :])
                    nc.vector.tensor_tensor(
                        ctx_sb[h * DV:(h + 1) * DV, p, :],
                        ctx_sb[h * DV:(h + 1) * DV, p, :], bc[0:DV, :],
                        OP.mult)
            return ctx_sb, last_mm

        def residual_add(get_in, nm):
            xn = pers.tile([P, ND, TOK], f32r, tag="x", bufs=2, name=f"x_{nm}")
            for m in range(ND):
                nc.vector.tensor_tensor(xn[:, m, :], get_in(m), x_cur[:, m, :],
                                        OP.add)
            return xn

        def ln_apply(xn, nm):
            """In-place layernorm of xn across the DM (partition-tiled) axis."""
            nonlocal x_cur
            with tc.tile_pool(name=f"lps_{nm}", bufs=1, space="PSUM") as lps:
                ssum = lps.tile([1, TOK], f32, tag="stsum", name=f"ssum_{nm}")
                ssq = lps.tile([1, TOK], f32, tag="stsq", name=f"ssq_{nm}")
                for m in range(ND):
                    sq = pers.tile([P, TOK], f32r, tag="sq", bufs=2,
                                   name=f"sq_{nm}{m}")
                    nc.scalar.square(sq[:], xn[:, m, :])
                    nc.tensor.matmul(ssum[:], ones_col[:], xn[:, m, :],
                                     start=(m == 0), stop=(m == ND - 1))
                    nc.tensor.matmul(ssq[:], ones_col[:], sq[:],
                                     start=(m == 0), stop=(m == ND - 1))
                mean = pers.tile([1, TOK], f32r, tag="mean", name=f"mean_{nm}")
                nc.vector.tensor_scalar_mul(mean[:], ssum[:], 1.0 / DM)
                es2 = pers.tile([1, TOK], f32, tag="es2", name=f"es2_{nm}")
                nc.vector.tensor_scalar_mul(es2[:], ssq[:], 1.0 / DM)
                msq = pers.tile([1, TOK], f32, tag="msq", name=f"msq_{nm}")
                nc.scalar.square(msq[:], mean[:])
                var = pers.tile([1, TOK], f32, tag="var", name=f"var_{nm}")
                nc.vector.tensor_tensor(var[:], es2[:], msq[:], OP.subtract)
                sS = pers.tile([1, TOK], f32r, tag="sS", name=f"sS_{nm}")
                nc.scalar.activation(sS[:], var[:], AF.Abs_reciprocal_sqrt,
                                     bias=eps_t[:])
                Mb = lps.tile([P, TOK], f32, tag="Mb", name=f"Mb_{nm}")
                nc.tensor.matmul(Mb[:], ones_row[:], mean[:], start=True,
                                 stop=True)
                Mbs = pers.tile([P, TOK], f32, tag="Mbs", name=f"Mbs_{nm}")
                nc.scalar.copy(Mbs[:], Mb[:])
                for m in range(ND):
                    nc.vector.tensor_tensor(xn[:, m, :], xn[:, m, :], Mbs[:],
                                            OP.subtract)
                Sb = lps.tile([P, TOK], f32, tag="Sb", name=f"Sb_{nm}")
                nc.tensor.matmul(Sb[:], ones_row[:], sS[:], start=True,
                                 stop=True)
                Sbs = pers.tile([P, TOK], f32, tag="Sbs", name=f"Sbs_{nm}")
                nc.scalar.copy(Sbs[:], Sb[:])
                for m in range(ND):
                    nc.vector.tensor_tensor(xn[:, m, :], xn[:, m, :], Sbs[:],
                                            OP.mult)
            x_cur = xn

        def wo_add(ph, aps, name, l, ctx_sb, nm):
            whs = [load_whb(ph, name, l, half, f"o{nm}{half}")
                   for half in range(2)]
            xn = pers.tile([P, ND, TOK], f32r, tag="x", bufs=2, name=f"x_{nm}")
            for m in range(ND):
                ps = aps.tile([P, TOK], f32, tag="proj", bufs=2,
                              name=f"wops_{nm}{m}")
                half, mm = divmod(m, 4)
                for v in range(ND):
                    nc.tensor.matmul(
                        ps[:], whs[half][:, v, mm * P:(mm + 1) * P],
                        ctx_sb[:, v, :], start=(v == 0), stop=(v == ND - 1))
                nc.vector.tensor_tensor(xn[:, m, :], ps[:], x_cur[:, m, :],
                                        OP.add)
            return xn

        for l in range(L):
            # cross K/V bounce buffers (computed in self phase, used in cross)
            ktb_c = dram.tile([DM, T], bf16, tag="ktbc", bufs=2,
                              name=f"ktbc_{l}")
            vab_c = dram.tile([T, H * (DV + 1)], bf16, tag="vabc", bufs=2,
                              name=f"vabc_{l}")

            with tc.tile_pool(name=f"ph1_{l}", bufs=1) as ph, \
                 tc.tile_pool(name=f"ps1_{l}", bufs=1, space="PSUM") as aps:
                if l == 0:
                    xb = ph.tile([P, ND, TOK], bf16, tag="xb", bufs=1,
                                 name="xb_s0")
                    nc.sync.dma_start(
                        xb[:], xT_ext.rearrange("(o p) t -> p o t", p=P))
                    xc0 = pers.tile([P, ND, TOK], f32r, tag="x", bufs=2,
                                    name="x0")
                    for m in range(ND):
                        nc.vector.tensor_copy(xc0[:, m, :], xb[:, m, :])
                    x_cur = xc0
                    xg_d = xg_d0
                else:
                    xb = cast_xb(ph, f"s{l}")
                    xb_d = dram.tile([DM, TOK], bf16, tag="xbd", bufs=2,
                                     name=f"xbd_{l}")
                    nc.sync.dma_start(
                        xb_d.rearrange("(o p) t -> p o t", p=P), xb[:])
                    xg_d = dram.tile([2 * DM, TOK], bf16, tag="xgd", bufs=2,
                                     name=f"xgd_{l}")
                    pair_ag(xb_d[:], xg_d, f"xg{l}")
                    for c in AG_SCHED[l]:
                        emit_wchunk(c)

                qt = q_proj(ph, aps, xb, "self_Wq", l, f"s{l}")

                def emit_cross_kv(after=None):
                    # cross K/V from enc (global token order); bounced to
                    # DRAM so they survive this pool scope. For l=0 this
                    # gap-fills the PE while the scalar engine works through
                    # the exps; for l>0 it hides the x pair-exchange.
                    kt_c = ph.tile([P, HP, 2, TOK], bf16, tag="ktoc",
                                   name=f"ktoc_{l}")
                    vg_c = ph.tile([P, NKB, H, DV + 1], bf16, tag="vaoc",
                                   name=f"vaoc_{l}")
                    kv_proj(ph, aps, "cross_Wk", "cross_Wv", l,
                            lambda d, s: enc_sb[:, d, s * TOK:(s + 1) * TOK],
                            lambda d, c: enc_sb[:, d, c * P:(c + 1) * P],
                            kt_c, vg_c, f"c{l}", after=after)
                    nc.sync.dma_start(
                        ktb_c.rearrange("(o p) (s t) -> p o s t", p=P, s=2),
                        kt_c[:])
                    nc.sync.dma_start(
                        vab_c.rearrange("(c p) z -> p c z", p=P)
                        .rearrange("p c (h v) -> p c h v", h=H), vg_c[:])

                if l > 0:
                    emit_cross_kv()

                # gathered x, layout [P, ND, s, TOK]; storage block c=s*4+j
                xg = ph.tile([P, ND, 2, TOK], bf16, tag="xg", bufs=1,
                             name=f"xg_{l}")
                for s in range(2):
                    nc.sync.dma_start(
                        xg[:, :, s, :],
                        xg_d[s * DM:(s + 1) * DM].rearrange(
                            "(o p) t -> p o t", p=P))

                kt_s = ph.tile([P, HP, 2, TOK], bf16, tag="kts",
                               name=f"kts_{l}")
                vg_s = ph.tile([P, NKB, H, DV + 1], bf16, tag="vgs",
                               name=f"vgs_{l}")
                kv_proj(ph, aps, "self_Wk", "self_Wv", l,
                        lambda d, s: xg[:, d, s, :],
                        lambda d, c: xg[:, d, c // 4,
                                        (c % 4) * P:(c % 4 + 1) * P],
                        kt_s, vg_s, f"s{l}")

                ctx, attn_last = attention(ph, aps, qt, kt_s, vg_s, self_causal, f"s{l}")

                xn = wo_add(ph, aps, "self_Wo", l, ctx, f"s{l}")
                if l == 0:
                    emit_cross_kv(after=attn_last)
            ln_apply(xn, f"s{l}")

            # cross sublayer
            with tc.tile_pool(name=f"ph4_{l}", bufs=1) as ph, \
                 tc.tile_pool(name=f"ps4_{l}", bufs=1, space="PSUM") as aps:
                ktg_c = ph.tile([P, HP, 2, TOK], bf16, tag="ktg",
                                name=f"ktg_{l}")
                nc.sync.dma_start(
                    ktg_c[:],
                    ktb_c.rearrange("(o p) (s t) -> p o s t", p=P, s=2))
                vgg_c = ph.tile([P, NKB, H, DV + 1], bf16, tag="vgg",
                                name=f"vgg_{l}")
                nc.sync.dma_start(
                    vgg_c[:],
                    vab_c.rearrange("(c p) z -> p c z", p=P)
                    .rearrange("p c (h v) -> p c h v", h=H))
                xb2 = cast_xb(ph, f"c{l}")
                qtc = q_proj(ph, aps, xb2, "cross_Wq", l, f"c{l}")
                ctx, _ = attention(ph, aps, qtc, ktg_c, vgg_c, False, f"c{l}")
                xn = wo_add(ph, aps, "cross_Wo", l, ctx, f"c{l}")
            ln_apply(xn, f"c{l}")

            # FFN
            with tc.tile_pool(name=f"ph6_{l}", bufs=1) as ph:
                xbf = cast_xb(ph, f"f{l}")
                h_sb = ph.tile([P, NF, TOK], bf16, tag="h", name=f"h_{l}")
                w1r = wview("ffn_W1", l)
                with tc.tile_pool(name=f"ps6_{l}", bufs=1,
                                  space="PSUM") as pools:
                    for c in range(DFF // TOK):
                        w1c = ph.tile([P, ND, TOK], bf16, tag="w1c", bufs=2,
                                      name=f"w1c_{l}{c}")
                        if c == 0:
                            # split first chunk across queues to cut latency
                            for d in range(ND):
                                nc.sync.dma_start(
                                    w1c[:, d, :], w1r[:, d, 0:TOK])
                        else:
                            nc.sync.dma_start(
                                w1c[:], w1r[:, :, c * TOK:(c + 1) * TOK])
                        for ft in range(4):
                            ps = pools.tile([P, TOK], f32, tag="hps", bufs=2,
                                            name=f"hps_{l}{c}{ft}")
                            for d in range(ND):
                                nc.tensor.matmul(
                                    ps[:], w1c[:, d, ft * P:(ft + 1) * P],
                                    xbf[:, d, :],
                                    start=(d == 0), stop=(d == ND - 1))
                            nc.scalar.activation(h_sb[:, c * 4 + ft, :],
                                                 ps[:], AF.Relu)
                w2r = wview("ffn_W2", l)
                with tc.tile_pool(name=f"ps7_{l}", bufs=1,
                                  space="PSUM") as pools:
                    yps = [pools.tile([P, TOK], f32, tag=f"y{m}",
                                      name=f"yps_{l}{m}") for m in range(ND)]
                    for f in range(NF):
                        w2f = ph.tile([P, DM], bf16, tag="w2f", bufs=3,
                                      name=f"w2f_{l}{f}")
                        nc.sync.dma_start(w2f[:], w2r[:, f, :])
                        for m in range(ND):
                            nc.tensor.matmul(
                                yps[m][:], w2f[:, m * P:(m + 1) * P],
                                h_sb[:, f, :],
                                start=(f == 0), stop=(f == NF - 1))
                    xn = residual_add(lambda m: yps[m][:], f"f{l}")
                ln_apply(xn, f"f{l}")

        # bf16 output halves the per-call zero-upload + result download
        yb = pers.tile([P, ND, TOK], bf16, tag="yb", name="yb")
        for m in range(ND):
            nc.vector.tensor_copy(yb[:, m, :], x_cur[:, m, :])
        yre = yT_ext.rearrange("(o p) t -> p o t", p=P)
        for m in range(ND):
            nc.sync.dma_start(yre[:, m, :], yb[:, m, :])

    nc.compile()
    return nc


def _get_built(self_causal=True):
    if self_causal not in _BUILT:
        _BUILT[self_causal] = _build(self_causal=self_causal)
    return _BUILT[self_causal]


def _host_shard(inputs):
    """Build per-core input maps from full inputs."""
    bf = ml_dtypes.bfloat16
    dec = np.asarray(inputs["dec_inputs"], dtype=np.float32)
    enc = np.asarray(inputs["enc_outputs"], dtype=np.float32)
    smask_full = np.asarray(inputs["dec_self_attn_mask"]).astype(bool)
    cmask = np.asarray(inputs["dec_enc_attn_mask"]).astype(bool)
    assert not cmask.any(), "kernel assumes open cross-attention mask"

    # canonical weight pack -> per-core 1/8 slices, chunk-aligned
    per_core = [[] for _ in range(8)]
    for chunk in CHUNKS:
        arrs = [np.asarray(inputs[n], np.float32)[l].astype(bf).ravel()
                for n, l in chunk]
        ch = np.concatenate(arrs).reshape(8, -1)
        for r in range(8):
            per_core[r].append(ch[r])
    wslices = [np.ascontiguousarray(np.concatenate(p)) for p in per_core]

    self_causal = smask_full.any()
    in_maps, row_sets = [], []
    for core in range(8):
        b, r = divmod(core, 2)
        rows = np.concatenate(
            [np.arange((2 * j + r) * P, (2 * j + r + 1) * P)
             for j in range(NTB)])
        row_sets.append((b, rows))
        xT = np.ascontiguousarray(dec[b][rows].T).astype(bf)
        encT = np.ascontiguousarray(
            enc[b].T[:, r * TOK:(r + 1) * TOK]).astype(bf)
        sm = np.ones((NKB, P, P), dtype=np.float32)
        mb = smask_full[b]
        if self_causal:
            for kb in range(NKB):
                qg0 = (2 * J0[kb] + r) * P
                blk = mb[qg0:qg0 + P, kb * P:(kb + 1) * P]     # [q, k]
                sm[kb] = (~blk.T).astype(np.float32)            # [k, q]
                for j in range(NTB):
                    qg = (2 * j + r) * P
                    bj = mb[qg:qg + P, kb * P:(kb + 1) * P]
                    if j < J0[kb]:
                        assert bj.all(), "skipped block not fully masked"
                    elif j > J0[kb]:
                        assert not bj.any(), \
                            "unmasked block outside computed window"
        in_map = {"xT": xT, "encT": encT, "smask": sm.astype(bf),
                  "wslice": wslices[core]}
        in_maps.append(in_map)
    return in_maps, row_sets, self_causal


def kernel(**inputs):
    from concourse.bass_utils import run_bass_kernel_spmd

    in_maps, row_sets, self_causal = _host_shard(inputs)
    nc = _get_built(self_causal)
    res = run_bass_kernel_spmd(nc, in_maps, core_ids=list(range(8)))
    out = np.empty((B, T, DM), dtype=np.float32)
    for core in range(8):
        b, rows = row_sets[core]
        out[b, rows, :] = np.asarray(res.results[core]["yT"],
                                     dtype=np.float32).T
    return out

